# revision 6
# baseline (speedup 1.0000x reference)
"""Trainium2 Bass kernel for nn_Criterion_73203422593120 (MTR-style loss).

Strategy (8 NeuronCores, SPMD single program):
  - dense-future loss (~189 MB of input): shard No=192 -> 24 per core;
    partition dim = Nc=256 (2 tiles of 128). Device computes per-(nc,no)
    masked GMM+vel sums, then per-core partial (reg*valid).sum(no) and
    valid.sum(no) vectors [256]. Host combines linearly.
  - layer (decoder) loss: shard T=80 -> 10 per core; partition dim = B=256.
    Device returns per-core partial sums over its t-shard: masked Laplace
    NLL, entropy-log, and masked ADE-dist, each [256, L*K]. Host does the
    tiny nonlinear epilogue (posterior softmax over K=6, max/min over K,
    KL, FDE at t=79) in float64 on [256,18] arrays.
  - goal loss: shard B -> 32 per core. Device returns min_kg(reg) and
    SmoothL1 partial sums [32, L]. Host combines.

All big-tensor FLOPs and all DMA-heavy streaming run on-device; host work
is O(B*K*L) on KB-sized partials.
"""

import sys

for _p in ("/opt/trn_rl_repo",):
    if _p not in sys.path:
        sys.path.insert(0, _p)

import numpy as np

import concourse.bacc as bacc
import concourse.tile as tile
from concourse import mybir
from concourse.bass_utils import run_bass_kernel_spmd

Act = mybir.ActivationFunctionType
Op = mybir.AluOpType
AX = mybir.AxisListType
dt32 = mybir.dt.float32

# problem dims
L, K, T, B, KG, NO, T2 = 3, 6, 80, 256, 64, 192, 80
ENTROPY_W, KL_W = 40.0, 20.0
GOAL_REG_W, DISP_W = 1.0, 0.5
LOG_STD_LO, LOG_STD_HI, RHO_LIM = -1.609, 5.0, 0.5
LOG_2PI = float(np.log(2.0 * np.pi))

M = 8           # cores
TS_ = T // M    # 10 t's per core (layer loss)
NOS = NO // M   # 24 no's per core (dense loss)
BS = B // M     # 32 b's per core (goal loss)
CH = 8          # dense no-chunk per inner tile
LK = L * K      # 18

_PROG = None


def _build_program(reps=1):
    nc = bacc.Bacc("TRN2", target_bir_lowering=False, debug=False)

    dpred = nc.dram_tensor("dpred", [B, NOS, T2, 7], dt32, kind="ExternalInput")
    dstate = nc.dram_tensor("dstate", [B, NOS, T2, 4], dt32, kind="ExternalInput")
    dmask = nc.dram_tensor("dmask", [B, NOS, T2], dt32, kind="ExternalInput")
    lpred = nc.dram_tensor("lpred", [L, K, TS_, B, 5], dt32, kind="ExternalInput")
    lgt = nc.dram_tensor("lgt", [B, TS_, 5], dt32, kind="ExternalInput")
    greg = nc.dram_tensor("greg", [L, KG, BS, 2], dt32, kind="ExternalInput")
    gfde = nc.dram_tensor("gfde", [L, BS, KG], dt32, kind="ExternalInput")
    ggt = nc.dram_tensor("ggt", [BS, 5], dt32, kind="ExternalInput")

    dense_out = nc.dram_tensor("dense_out", [B, 2], dt32, kind="ExternalOutput")
    nll_out = nc.dram_tensor("nll_out", [B, LK], dt32, kind="ExternalOutput")
    ent_out = nc.dram_tensor("ent_out", [B, LK], dt32, kind="ExternalOutput")
    ade_out = nc.dram_tensor("ade_out", [B, LK], dt32, kind="ExternalOutput")
    goal_out = nc.dram_tensor("goal_out", [BS, 2 * L], dt32, kind="ExternalOutput")

    v = nc.vector
    a = nc.scalar

    with tile.TileContext(nc) as tc:
        with tc.tile_pool(name="consts", bufs=1) as cpool:
            lnhalf = cpool.tile([128, 1], dt32)
            v.memset(lnhalf, float(np.log(0.5)))
            ln2 = cpool.tile([128, 1], dt32)
            v.memset(ln2, float(np.log(2.0)))

            for r in range(reps):
                _goal_part(nc, tc, v, a, greg, gfde, ggt, goal_out, r)
                _layer_part(nc, tc, v, a, lpred, lgt, nll_out, ent_out, ade_out, ln2, r)
                _dense_part(nc, tc, v, a, dpred, dstate, dmask, dense_out, lnhalf, r)

    nc.compile()
    return nc


def _goal_part(nc, tc, v, a, greg, gfde, ggt, goal_out, r=0):
    P = BS  # 32 partitions
    with tc.tile_pool(name=f"goal{r}", bufs=1) as pool:
        gp = pool.tile([P, L, KG, 2], dt32)
        for l in range(L):
            # src [KG, BS, 2] -> [b, kg, c]
            nc.sync.dma_start(out=gp[:, l], in_=greg.ap()[l].transpose([1, 0, 2]))
        gf = pool.tile([P, L, KG], dt32)
        nc.sync.dma_start(out=gf, in_=gfde.ap().transpose([1, 0, 2]))
        gg = pool.tile([P, 5], dt32)
        nc.sync.dma_start(out=gg, in_=ggt.ap())

        # diff = goal_reg - gt_xy  (broadcast over l, kg); norm over c
        gd = pool.tile([P, L, KG, 2], dt32)
        gg_b = gg[:, 0:2].unsqueeze(1).unsqueeze(1).broadcast_to([P, L, KG, 2])
        v.tensor_tensor(out=gd, in0=gp, in1=gg_b, op=Op.subtract)
        a.activation(out=gd, in_=gd, func=Act.Square)
        gs = pool.tile([P, L, KG], dt32)
        v.tensor_tensor(out=gs, in0=gd[:, :, :, 0], in1=gd[:, :, :, 1], op=Op.add)
        a.activation(out=gs, in_=gs, func=Act.Sqrt)
        # reg = norm * mask_last (per-partition scalar)
        v.tensor_scalar(out=gs, in0=gs, scalar1=gg[:, 4:5], scalar2=None, op0=Op.mult)

        stage = pool.tile([P, 2 * L], dt32)
        # min over kg per l
        v.tensor_reduce(out=stage[:, 0:L], in_=gs, axis=AX.X, op=Op.min)
        # smooth-l1 partial: d = |gfde - reg|; y = 0.5*min(d,1)^2 + max(d,1)
        dd = pool.tile([P, L, KG], dt32)
        v.tensor_tensor(out=dd, in0=gf, in1=gs, op=Op.subtract)
        a.activation(out=dd, in_=dd, func=Act.Abs)
        m1 = pool.tile([P, L, KG], dt32)
        v.tensor_scalar(out=m1, in0=dd, scalar1=1.0, scalar2=None, op0=Op.min)
        a.activation(out=m1, in_=m1, func=Act.Square, scale=float(np.sqrt(0.5)))
        v.tensor_scalar(out=dd, in0=dd, scalar1=1.0, scalar2=None, op0=Op.max)
        v.tensor_tensor(out=dd, in0=dd, in1=m1, op=Op.add)
        v.tensor_reduce(out=stage[:, L:2 * L], in_=dd, axis=AX.X, op=Op.add)
        nc.sync.dma_start(out=goal_out.ap(), in_=stage)


def _layer_part(nc, tc, v, a, lpred, lgt, nll_out, ent_out, ade_out, ln2, r=0):
    with tc.tile_pool(name=f"layer_in{r}", bufs=2) as ipool, \
         tc.tile_pool(name=f"layer_tmp{r}", bufs=2) as tpool, \
         tc.tile_pool(name=f"layer_out{r}", bufs=2) as opool:
        for h in range(2):
            rows = slice(h * 128, (h + 1) * 128)
            pt = ipool.tile([128, LK, TS_, 5], dt32, tag="lpred")
            for l in range(L):
                # src [K, TS_, B, 5] -> [b, k, t, c]
                src = lpred.ap()[l].transpose([2, 0, 1, 3])[rows]
                nc.sync.dma_start(out=pt[:, l * K:(l + 1) * K], in_=src)
            gt_ = ipool.tile([128, TS_, 5], dt32, tag="lgt")
            nc.sync.dma_start(out=gt_, in_=lgt.ap()[rows])

            data_b = gt_[:, :, 0:2].unsqueeze(1).broadcast_to([128, LK, TS_, 2])
            mask_b = gt_[:, :, 4].unsqueeze(1).broadcast_to([128, LK, TS_])

            def t4(tag):
                return tpool.tile([128, LK, TS_, 2], dt32, tag=tag, name=tag)

            def t3(tag):
                return tpool.tile([128, LK, TS_], dt32, tag=tag, name=tag)

            # ---- laplace nll ----
            et = t4("et")
            v.tensor_tensor(out=et, in0=data_b, in1=pt[:, :, :, 0:2], op=Op.subtract)
            ae = t4("ae")
            a.activation(out=ae, in_=et, func=Act.Abs)
            lnb = t4("lnb")
            a.activation(out=lnb, in_=pt[:, :, :, 2:4], func=Act.Ln, scale=2.0)
            rb = t4("rb")
            a.activation(out=rb, in_=lnb, func=Act.Exp, scale=-1.0, bias=ln2)
            v.tensor_tensor(out=ae, in0=ae, in1=rb, op=Op.mult)
            v.tensor_tensor(out=ae, in0=ae, in1=lnb, op=Op.add)
            nt = t3("nt")
            v.tensor_tensor(out=nt, in0=ae[:, :, :, 0], in1=ae[:, :, :, 1], op=Op.add)
            v.tensor_tensor(out=nt, in0=nt, in1=mask_b, op=Op.mult)
            stage = opool.tile([128, LK], dt32, tag="nll_stage")
            v.tensor_reduce(out=stage, in_=nt, axis=AX.X, op=Op.add)
            nc.sync.dma_start(out=nll_out.ap()[rows], in_=stage)

            # ---- entropy log-term: ln((sx*sy)^2 * (1-rho^2)) ----
            ss = t3("ss")
            v.tensor_tensor(out=ss, in0=pt[:, :, :, 2], in1=pt[:, :, :, 3], op=Op.mult)
            a.activation(out=ss, in_=ss, func=Act.Square)
            rq = t3("rq")
            a.activation(out=rq, in_=pt[:, :, :, 4], func=Act.Square)
            a.activation(out=rq, in_=rq, func=Act.Copy, scale=-1.0, bias=1.0)
            v.tensor_tensor(out=ss, in0=ss, in1=rq, op=Op.mult)
            a.activation(out=ss, in_=ss, func=Act.Ln)
            stage2 = opool.tile([128, LK], dt32, tag="ent_stage")
            v.tensor_reduce(out=stage2, in_=ss, axis=AX.X, op=Op.add)
            nc.sync.dma_start(out=ent_out.ap()[rows], in_=stage2)

            # ---- masked ade dist ----
            e2 = t4("e2")
            a.activation(out=e2, in_=et, func=Act.Square)
            sd = t3("sd")
            v.tensor_tensor(out=sd, in0=e2[:, :, :, 0], in1=e2[:, :, :, 1], op=Op.add)
            a.activation(out=sd, in_=sd, func=Act.Sqrt)
            v.tensor_tensor(out=sd, in0=sd, in1=mask_b, op=Op.mult)
            stage3 = opool.tile([128, LK], dt32, tag="ade_stage")
            v.tensor_reduce(out=stage3, in_=sd, axis=AX.X, op=Op.add)
            nc.sync.dma_start(out=ade_out.ap()[rows], in_=stage3)


def _dense_part(nc, tc, v, a, dpred, dstate, dmask, dense_out, lnhalf, r=0):
    nch = NOS // CH
    with tc.tile_pool(name=f"dense_in{r}", bufs=2) as ipool, \
         tc.tile_pool(name=f"dense_tmp{r}", bufs=2) as tpool, \
         tc.tile_pool(name=f"dense_acc{r}", bufs=2) as apool:
        for h in range(2):
            rows = slice(h * 128, (h + 1) * 128)
            reg_all = apool.tile([128, NOS], dt32, tag="reg_all")
            msum_all = apool.tile([128, NOS], dt32, tag="msum_all")
            for ci in range(nch):
                nsl = slice(ci * CH, (ci + 1) * CH)
                p = ipool.tile([128, CH, T2, 7], dt32, tag="dpred")
                s = ipool.tile([128, CH, T2, 4], dt32, tag="dstate")
                m = ipool.tile([128, CH, T2], dt32, tag="dmask")
                nc.sync.dma_start(out=p, in_=dpred.ap()[rows, nsl])
                nc.sync.dma_start(out=s, in_=dstate.ap()[rows, nsl])
                nc.sync.dma_start(out=m, in_=dmask.ap()[rows, nsl])

                def tmp(tag):
                    return tpool.tile([128, CH, T2], dt32, tag=tag, name=tag)

                # vel = |p5-s2| + |p6-s3|
                ta = tmp("ta")
                v.tensor_tensor(out=ta, in0=p[:, :, :, 5], in1=s[:, :, :, 2], op=Op.subtract)
                a.activation(out=ta, in_=ta, func=Act.Abs)
                tb = tmp("tb")
                v.tensor_tensor(out=tb, in0=p[:, :, :, 6], in1=s[:, :, :, 3], op=Op.subtract)
                a.activation(out=tb, in_=tb, func=Act.Abs)
                v.tensor_tensor(out=ta, in0=ta, in1=tb, op=Op.add)

                # clipped log-stds, coef base, inverse scales
                ls1 = tmp("ls1")
                v.tensor_scalar(out=ls1, in0=p[:, :, :, 2], scalar1=LOG_STD_LO,
                                scalar2=LOG_STD_HI, op0=Op.max, op1=Op.min)
                ls2 = tmp("ls2")
                v.tensor_scalar(out=ls2, in0=p[:, :, :, 3], scalar1=LOG_STD_LO,
                                scalar2=LOG_STD_HI, op0=Op.max, op1=Op.min)
                co = tmp("co")
                v.tensor_tensor(out=co, in0=ls1, in1=ls2, op=Op.add)
                a.activation(out=ls1, in_=ls1, func=Act.Exp, scale=-1.0)  # 1/s1
                a.activation(out=ls2, in_=ls2, func=Act.Exp, scale=-1.0)  # 1/s2

                # u = dx/s1, v = dy/s2
                dx = tmp("dx")
                v.tensor_tensor(out=dx, in0=s[:, :, :, 0], in1=p[:, :, :, 0], op=Op.subtract)
                dy = tmp("dy")
                v.tensor_tensor(out=dy, in0=s[:, :, :, 1], in1=p[:, :, :, 1], op=Op.subtract)
                v.tensor_tensor(out=dx, in0=dx, in1=ls1, op=Op.mult)
                v.tensor_tensor(out=dy, in0=dy, in1=ls2, op=Op.mult)
                uv = tmp("uv")
                v.tensor_tensor(out=uv, in0=dx, in1=dy, op=Op.mult)
                a.activation(out=dx, in_=dx, func=Act.Square)
                a.activation(out=dy, in_=dy, func=Act.Square)
                v.tensor_tensor(out=dx, in0=dx, in1=dy, op=Op.add)  # u^2+v^2

                # rho terms
                rho = tmp("rho")
                v.tensor_scalar(out=rho, in0=p[:, :, :, 4], scalar1=-RHO_LIM,
                                scalar2=RHO_LIM, op0=Op.max, op1=Op.min)
                v.tensor_scalar(out=tb, in0=rho, scalar1=2.0, scalar2=None, op0=Op.mult)
                v.tensor_tensor(out=uv, in0=tb, in1=uv, op=Op.mult)   # 2*rho*u*v
                v.tensor_tensor(out=dx, in0=dx, in1=uv, op=Op.subtract)  # W
                a.activation(out=rho, in_=rho, func=Act.Square)
                a.activation(out=rho, in_=rho, func=Act.Copy, scale=-1.0, bias=1.0)  # 1-rho^2
                a.activation(out=rho, in_=rho, func=Act.Sqrt)
                a.activation(out=rho, in_=rho, func=Act.Ln)            # hl = 0.5*ln(1-rho^2)
                a.activation(out=tb, in_=rho, func=Act.Exp, scale=-2.0, bias=lnhalf)  # 0.5/(1-rho^2)
                v.tensor_tensor(out=dx, in0=tb, in1=dx, op=Op.mult)    # expt

                # z = vel + ls1c+ls2c + hl + expt, masked
                v.tensor_tensor(out=ta, in0=ta, in1=co, op=Op.add)
                v.tensor_tensor(out=ta, in0=ta, in1=rho, op=Op.add)
                v.tensor_tensor(out=ta, in0=ta, in1=dx, op=Op.add)
                v.tensor_tensor(out=ta, in0=ta, in1=m, op=Op.mult)

                v.tensor_reduce(out=reg_all[:, nsl], in_=ta, axis=AX.X, op=Op.add)
                v.tensor_reduce(out=msum_all[:, nsl], in_=m, axis=AX.X, op=Op.add)

            valid = apool.tile([128, NOS], dt32, tag="valid")
            v.tensor_scalar(out=valid, in0=msum_all, scalar1=0.0, scalar2=None, op0=Op.is_gt)
            v.tensor_tensor(out=reg_all, in0=reg_all, in1=valid, op=Op.mult)
            stage = apool.tile([128, 2], dt32, tag="dstage")
            v.tensor_reduce(out=stage[:, 0:1], in_=reg_all, axis=AX.X, op=Op.add)
            v.tensor_reduce(out=stage[:, 1:2], in_=valid, axis=AX.X, op=Op.add)
            nc.sync.dma_start(out=dense_out.ap()[rows], in_=stage)


def _run_device(inputs, **run_kwargs):
    global _PROG
    if _PROG is None:
        _PROG = _build_program()
    preds = inputs["preds"]
    gt = inputs["gt_decoder"]
    in_maps = []
    for c in range(M):
        in_maps.append({
            "dpred": np.ascontiguousarray(inputs["dense_future_pred"][:, c * NOS:(c + 1) * NOS]),
            "dstate": np.ascontiguousarray(inputs["obj_trajs_future_state"][:, c * NOS:(c + 1) * NOS]),
            "dmask": np.ascontiguousarray(inputs["obj_trajs_future_mask"][:, c * NOS:(c + 1) * NOS]),
            "lpred": np.ascontiguousarray(preds[:, :, c * TS_:(c + 1) * TS_]),
            "lgt": np.ascontiguousarray(gt[:, c * TS_:(c + 1) * TS_]),
            "greg": np.ascontiguousarray(inputs["goal_reg"][:, :, c * BS:(c + 1) * BS]),
            "gfde": np.ascontiguousarray(inputs["goal_FDE"][:, c * BS:(c + 1) * BS]),
            "ggt": np.ascontiguousarray(gt[c * BS:(c + 1) * BS, T - 1]),
        })
    res = run_bass_kernel_spmd(_PROG, in_maps, core_ids=list(range(M)), **run_kwargs)
    return res


def _epilogue(inputs, results):
    f64 = np.float64
    preds = np.asarray(inputs["preds"], dtype=f64)
    gt = np.asarray(inputs["gt_decoder"], dtype=f64)
    modes = np.asarray(inputs["modes_preds"], dtype=f64)

    nll = sum(r["nll_out"].astype(f64) for r in results).reshape(B, L, K)
    entp = sum(r["ent_out"].astype(f64) for r in results).reshape(B, L, K)
    adep = sum(r["ade_out"].astype(f64) for r in results).reshape(B, L, K)

    mask_last = gt[:, -1, -1]  # [B]
    total = 0.0
    for l in range(L):
        nll_l = nll[:, l]                     # [B,K]
        # posterior over modes: float32 softmax to mirror the fp32 reference
        # exactly (incl. underflow-to-zero -> log(0) -> NaN in the KL term)
        f32 = np.float32
        modes32 = modes[l].astype(f32)
        lp = (-nll_l).astype(f32) + np.log(modes32)
        lp = lp - lp.max(-1, keepdims=True)
        with np.errstate(divide="ignore", invalid="ignore", under="ignore"):
            post32 = np.exp(lp)
            post32 = post32 / post32.sum(-1, keepdims=True)
            kl = KL_W * f64(
                (post32 * (np.log(post32) - np.log(modes32))).sum(dtype=f32)) / B
        post = post32.astype(f64)
        loss = (nll_l * post).sum(-1).mean()
        ent_bk = (1.0 + LOG_2PI) * T + 0.5 * entp[:, l]
        ent_loss = ent_bk.max(-1).mean()
        # fde at t = T-1 (host: tiny)
        pf = preds[l, :, -1, :, :2].transpose(1, 0, 2)       # [B,K,2]
        fde = np.linalg.norm(pf - gt[:, None, -1, :2], axis=-1) * mask_last[:, None]
        ade = adep[:, l] / T
        adefde = 100.0 * (fde + ade).min(-1).mean()
        total += loss + ENTROPY_W * ent_loss + kl + adefde
    total /= L

    # goal loss
    regmin = np.concatenate([r["goal_out"][:, 0:L].astype(f64) for r in results], 0)   # [B,L]
    smsum = np.concatenate([r["goal_out"][:, L:2 * L].astype(f64) for r in results], 0)
    for l in range(L):
        disp = smsum[:, l].sum() / (B * KG) - 1.0
        total += GOAL_REG_W * regmin[:, l].mean() + DISP_W * disp

    # dense loss
    regv = sum(r["dense_out"][:, 0].astype(f64) for r in results)
    vcnt = sum(r["dense_out"][:, 1].astype(f64) for r in results)
    per_center = regv / np.maximum(vcnt, 1.0)
    total += per_center.mean()
    return np.float32(total)


def kernel(**inputs):
    res = _run_device(inputs)
    return _epilogue(inputs, res.results)


if __name__ == "__main__":
    rng = np.random.default_rng(0)
    demo = {
        "modes_preds": np.abs(rng.standard_normal((L, B, K))).astype(np.float32) + 0.1,
        "preds": rng.standard_normal((L, K, T, B, 5)).astype(np.float32),
        "gt_decoder": rng.standard_normal((B, T, 5)).astype(np.float32),
        "goal_reg": rng.standard_normal((L, KG, B, 2)).astype(np.float32),
        "goal_FDE": rng.standard_normal((L, B, KG)).astype(np.float32),
        "dense_future_pred": rng.standard_normal((B, NO, T2, 7)).astype(np.float32),
        "obj_trajs_future_state": rng.standard_normal((B, NO, T2, 4)).astype(np.float32),
        "obj_trajs_future_mask": (rng.random((B, NO, T2)) < 0.7).astype(np.float32),
    }
    print(kernel(**demo))
